# revision 1
# baseline (speedup 1.0000x reference)
"""Trainium2 Bass kernel for nn_Colorizer (retrieval_knn).

Pipeline (per sample, data-parallel over N=8 samples -> 8 cores):
  1. Patch-embed conv as matmul: featsT[c, p] = W[k, c]^T @ patchesT[k, p]
     (k = 8*8*3 = 192 patch pixels, p = 4 images * 32*32 patches = 4096)
  2. Similarity S[r, t] = refT[c, r]^T @ tgtT[c, t]   (r = 3072, t = 1024)
  3. E = exp(S - 50)  (softmax over r is shift-invariant; max|S| ~= 87 so
     the constant shift prevents fp32 exp overflow; underflow to 0 is safe)
  4. predT_unnorm = labels_aug^T @ E with labels_aug = [ones(16),
     zeros(16), labels(16)]: rows 0..15 = replicated softmax
     denominator, rows 32..47 = unnormalized predictions (zeros keep
     the blocks 32-partition-aligned; custom-DVE reciprocal requires
     partition base 0, standard ops handle base 32)
  5. Normalize: out = pred_rows * reciprocal(denom_rows), DMA out as
     [16, 1024]; host transposes to [1024, 16].

Host side only reshapes/transposes data (im2col layout + sharding); all
FLOPs run on device. Matmuls use float32r (TF32-like, full PE rate).

Perf notes (measured on HW):
  - PE clock needs ~5 us of sustained matmul work to leave the HAM
    throttle (1.2 -> 2.4 GHz); bf16 warm-up matmuls run during the DMA
    prologue so the real work starts warm.
  - pred matmuls are emitted two chunks behind their exp so the PE
    never waits on ACT (in-order PE queue would otherwise bubble).
  - conv PSUM->SBUF casts alternate DVE/ACT so the cast never gates PE.

Built on bacc.Bacc so compile() legalizes multi-semaphore waits (TRN2
instructions accept only one sync wait).
"""

import numpy as np

import concourse.mybir as mybir
from concourse import bacc
from concourse.bass_utils import run_bass_kernel_spmd
from concourse.tile import TileContext

F32 = mybir.dt.float32
F32R = mybir.dt.float32r
BF16 = mybir.dt.bfloat16

N = 8            # samples == cores
R_T, T_T = 3, 1  # ref / target frames
H = W_IMG = 256
C = 3
PATCH = 8
FEAT = 256
K_LAB = 16
HP = H // PATCH          # 32
PPI = HP * HP            # 1024 patches per image
NIMG = R_T + T_T         # 4
NPAT = NIMG * PPI        # 4096
KPIX = PATCH * PATCH * C  # 192
KPAD = 256               # K padded to 2x128 (K=64 matmuls run ~3x slow)
R = R_T * PPI            # 3072
T = T_T * PPI            # 1024
RC = R // 128            # 24 r-chunks
LABC = 48                # 16 ones cols, 16 zero cols, 16 label cols
LABW = RC * LABC         # swizzled label columns
EXP_SHIFT = -50.0
N_WARMUP = 18


def _build_nc():
    nc = bacc.Bacc(trn_type="TRN2", target_bir_lowering=False)

    pt_d = nc.declare_dram_parameter("pt", [KPIX, NPAT], F32R, isOutput=False)
    w_d = nc.declare_dram_parameter("w", [KPAD, FEAT], F32R, isOutput=False)
    lab_d = nc.declare_dram_parameter("lab", [128, RC * K_LAB], F32R, isOutput=False)
    out_d = nc.declare_dram_parameter("out", [K_LAB, T], F32, isOutput=True)

    with TileContext(nc) as tc:
        with (
            tc.tile_pool(name="const", bufs=1) as const,
            tc.tile_pool(name="feats", bufs=1) as feats,
            tc.tile_pool(name="mmps", bufs=2, space="PSUM") as mmps,
            tc.tile_pool(name="predps", bufs=1, space="PSUM") as predps,
            tc.tile_pool(name="wps", bufs=2, space="PSUM") as wpsp,
            tc.tile_pool(name="epool", bufs=4) as epool,
            tc.tile_pool(name="opool", bufs=2) as opool,
        ):
            # PE warm-up source: first DVE op so matmuls can start early
            wu_sb = const.tile([128, 512], BF16, tag="wu")
            nc.vector.memset(wu_sb, 0.0)

            # ---- input loads: 2 HWDGE rings (sync + scalar), tgt first ----
            w_sb0 = const.tile([128, FEAT], F32R, tag="w0")
            w_sb1 = const.tile([KPAD - 128, FEAT], F32R, tag="w1")
            nc.sync.dma_start(out=w_sb0, in_=w_d.ap()[0:128, :])
            nc.gpsimd.dma_start(out=w_sb1, in_=w_d.ap()[128:KPAD, :])

            shift_sb = const.tile([128, 1], F32, tag="shift")
            nc.vector.memset(shift_sb, EXP_SHIFT)

            pt_sb0 = const.tile([128, NPAT], F32R, tag="pt0")
            pt_sb1 = const.tile([KPAD - 128, NPAT], F32R, tag="pt1")
            # pad rows 64..127 of the K=128..255 tile with zeros on-chip
            nc.vector.memset(pt_sb1[64:128, :].bitcast(F32), 0.0)
            NB_ORDER = (3, 0, 1, 2)  # tgt image block first
            NBD = 4
            for nb in NB_ORDER:
                sl = slice(nb * (NPAT // NBD), (nb + 1) * (NPAT // NBD))
                nc.sync.dma_start(out=pt_sb0[:, sl], in_=pt_d.ap()[0:128, sl])
                nc.gpsimd.dma_start(
                    out=pt_sb1[0:KPIX - 128, sl], in_=pt_d.ap()[128:KPIX, sl]
                )

            lab_sb = const.tile([128, RC, 48], F32R, tag="lab")
            nc.gpsimd.memset(lab_sb[:, :, 0:16].bitcast(F32), 1.0)
            nc.gpsimd.memset(lab_sb[:, :, 16:32].bitcast(F32), 0.0)
            nc.gpsimd.dma_start(
                out=lab_sb[:, :, 32:48],
                in_=lab_d.ap().rearrange("p (rc k) -> p rc k", k=K_LAB),
            )

            # ---- PE clock warm-up during the DMA prologue (HAM) ----
            for _ in range(N_WARMUP):
                wps = wpsp.tile([128, 512], F32, tag="wp", name="wps")
                nc.tensor.matmul(wps, wu_sb[:, 0:128], wu_sb, start=True, stop=True)

            # ---- 1. conv: featsT[c, p] (c split in two 128-row tiles) ----
            f_sb = [
                feats.tile([128, NPAT], F32R, tag="f0", name="f_sb0"),
                feats.tile([128, NPAT], F32R, tag="f1", name="f_sb1"),
            ]
            NB = 4  # column blocks of 1024
            BW = NPAT // NB
            pred_ps = predps.tile([LABC, T], F32, tag="pred")

            def conv_block(nb):
                for cc in range(2):
                    ps = mmps.tile([128, BW], F32, tag="mm", name="ps")
                    csl = slice(cc * 128, (cc + 1) * 128)
                    for h in range(2):
                        hsl = slice(nb * BW + h * 512, nb * BW + (h + 1) * 512)
                        psl = slice(h * 512, (h + 1) * 512)
                        nc.tensor.matmul(
                            ps[:, psl], w_sb0[:, csl], pt_sb0[:, hsl],
                            start=True, stop=False,
                        )
                        nc.tensor.matmul(
                            ps[:, psl], w_sb1[:, csl], pt_sb1[:, hsl],
                            start=False, stop=True,
                        )
                    dst = f_sb[cc][:, nb * BW:(nb + 1) * BW]
                    # split the cast across DVE and ACT so neither gates PE
                    nc.vector.tensor_copy(dst[:, 0:512], ps[:, 0:512])
                    nc.scalar.copy(dst[:, 512:BW], ps[:, 512:BW])

            e_tiles = {}

            def s_part(rc):
                rsl = slice(rc * 128, (rc + 1) * 128)
                s_ps = mmps.tile([128, T], F32, tag="mm", name="s_ps")
                for th in range(2):
                    psl = slice(th * 512, (th + 1) * 512)
                    tsl = slice(R + th * 512, R + (th + 1) * 512)
                    nc.tensor.matmul(
                        s_ps[:, psl], f_sb[0][:, rsl], f_sb[0][:, tsl],
                        start=True, stop=False,
                    )
                    nc.tensor.matmul(
                        s_ps[:, psl], f_sb[1][:, rsl], f_sb[1][:, tsl],
                        start=False, stop=True,
                    )
                e_sb = epool.tile([128, T], F32R, tag="e", name="e_sb")
                nc.scalar.activation(
                    e_sb, s_ps, mybir.ActivationFunctionType.Exp,
                    bias=shift_sb, scale=1.0,
                )
                e_tiles[rc] = e_sb

            def pred_part(rc):
                e_sb = e_tiles.pop(rc)
                for th in range(2):
                    psl = slice(th * 512, (th + 1) * 512)
                    nc.tensor.matmul(
                        pred_ps[:, psl],
                        lab_sb[:, rc, :],
                        e_sb[:, psl],
                        start=(rc == 0), stop=(rc == RC - 1),
                    )

            # conv blocks feed S chunks; pred lags two chunks behind its exp
            PRED_LAG = 2
            emitted = []

            def emit_s(rc):
                s_part(rc)
                emitted.append(rc)
                if len(emitted) > PRED_LAG:
                    pred_part(emitted[len(emitted) - 1 - PRED_LAG])

            conv_block(3)
            conv_block(0)
            for rc in range(0, 8):
                emit_s(rc)
            conv_block(1)
            for rc in range(8, 16):
                emit_s(rc)
            conv_block(2)
            for rc in range(16, 24):
                emit_s(rc)
            for rc in emitted[-PRED_LAG:]:
                pred_part(rc)

            # ---- 5. normalize label rows by replicated denom rows ----
            rec = opool.tile([K_LAB, T], F32, tag="rec")
            nc.vector.reciprocal_approx_fast(rec, pred_ps[0:K_LAB, :])
            o_sb = opool.tile([K_LAB, T], F32, tag="o")
            nc.vector.tensor_mul(o_sb, pred_ps[32:32 + K_LAB, :], rec)
            nc.sync.dma_start(out=out_d.ap(), in_=o_sb)

    nc.compile()
    return nc


_NC_CACHE = None


def _get_nc():
    global _NC_CACHE
    if _NC_CACHE is None:
        _NC_CACHE = _build_nc()
    return _NC_CACHE


def prep_in_maps(reference_images, target_images, reference_labels, w_feat):
    """Host-side sharding + layout prep (no arithmetic)."""
    ri = np.ascontiguousarray(reference_images, dtype=np.float32)
    ti = np.ascontiguousarray(target_images, dtype=np.float32)
    lab = np.ascontiguousarray(reference_labels, dtype=np.float32)
    wf = np.ascontiguousarray(w_feat, dtype=np.float32)

    w2 = np.zeros((KPAD, FEAT), np.float32)
    w2[:KPIX] = wf.reshape(KPIX, FEAT)
    imgs = np.concatenate([ri, ti], axis=1)  # [N, 4, H, W, C]
    # patchesT[n] : [(dy dx ch), (img py px)]
    ptT = np.ascontiguousarray(
        imgs.reshape(N, NIMG, HP, PATCH, HP, PATCH, C)
        .transpose(0, 3, 5, 6, 1, 2, 4)
        .reshape(N, KPIX, NPAT)
    )
    lab_sw = np.ascontiguousarray(
        lab.reshape(N, RC, 128, K_LAB).transpose(0, 2, 1, 3).reshape(N, 128, RC * K_LAB)
    )
    return [
        {"pt": ptT[n], "w": w2, "lab": lab_sw[n]} for n in range(N)
    ]


def run(in_maps, **kwargs):
    nc = _get_nc()
    return run_bass_kernel_spmd(nc, in_maps, list(range(N)), **kwargs)


def kernel(reference_images, target_images, reference_labels, w_feat):
    in_maps = prep_in_maps(
        reference_images, target_images, reference_labels, w_feat
    )
    res = run(in_maps)
    # device emits [16, T]; transpose to [T, 16] here (pure layout)
    out = np.stack(
        [np.ascontiguousarray(res.results[n]["out"].T) for n in range(N)]
    )
    return out.reshape(N, T_T, HP, HP, K_LAB)



# revision 3
# speedup vs baseline: 1.0891x; 1.0891x over previous
"""Trainium2 Bass kernel for nn_Colorizer (retrieval_knn).

v2 — Gram-matrix reformulation + all-bf16 (validated rel-err ~6e-3 vs
the 2e-2 gate on the fixed setup_inputs seed):

  S[r,t] = (W^T p_r)·(W^T p_t) = p_r^T (W W^T) p_t
so the reference-side conv disappears entirely:
  1. G = W W^T            [192,192]   (768 PE cycles)
  2. gt = G @ p_tgt       [192,1024]  (4096 cycles; replaces the 16384-
     cycle 4-image conv of the v1 kernel)
  3. S chunks: S[rc] = p_ref[:,rc]^T @ gt   (24 x 2048 cycles)
  4. E = exp(S - 50) on ACT (shift keeps fp32 exp in range; softmax is
     shift-invariant, underflow to 0 is harmless), output bf16
  5. pred rows via lab_aug = [ones;zeros;labels] matmul (denominator
     rides along as rows 0..15), normalize, DMA out [16,1024] fp32.

All matmuls bf16 (1 col/cycle at full clock). All inputs land as bf16,
halving HBM traffic vs fp32 (1.8 MB/core); loads are spread over four
DGE rings (sync/scalar/gpsimd/vector) so the first S chunk is ready
~13.5 us in. PE work: 768 + 4096 + 24*(2048 S + 1024 pred) = ~78.6k
cycles. Warm-up matmuls during the DMA prologue start the HAM clock
ramp (1.2 -> 2.4 GHz needs ~3 us of continuous PE work).

Zero-padding: contract chunks beyond k=128 use rows 64:128 zeroed on
BOTH operands (0 x garbage could be NaN).

Host side only reshapes/transposes/casts data; all FLOPs run on device.
Built on bacc.Bacc so compile() legalizes multi-semaphore waits.
"""

import numpy as np
from ml_dtypes import bfloat16

import concourse.mybir as mybir
from concourse import bacc
from concourse.bass_utils import run_bass_kernel_spmd
from concourse.tile import TileContext

F32 = mybir.dt.float32
BF16 = mybir.dt.bfloat16

N = 8            # samples == cores
R_T, T_T = 3, 1  # ref / target frames
H = W_IMG = 256
C = 3
PATCH = 8
FEAT = 256
K_LAB = 16
HP = H // PATCH          # 32
PPI = HP * HP            # 1024 patches per image
NIMG = R_T + T_T         # 4
NPAT = NIMG * PPI        # 4096
KPIX = PATCH * PATCH * C  # 192
R = R_T * PPI            # 3072
T = T_T * PPI            # 1024
RC = R // 128            # 24 r-chunks
LABC = 48                # 16 ones cols, 16 zero cols, 16 label cols
EXP_SHIFT = -50.0
N_WARMUP = 6


def _build_nc():
    nc = bacc.Bacc(trn_type="TRN2", target_bir_lowering=False)

    pt0_d = nc.declare_dram_parameter("pt0", [128, NPAT], BF16, isOutput=False)
    pt1_d = nc.declare_dram_parameter("pt1", [64, NPAT], BF16, isOutput=False)
    wt_d = nc.declare_dram_parameter("wt", [FEAT, KPIX], BF16, isOutput=False)
    lab_d = nc.declare_dram_parameter("lab", [128, RC * K_LAB], BF16, isOutput=False)
    out_d = nc.declare_dram_parameter("out", [K_LAB, T], F32, isOutput=True)

    with TileContext(nc) as tc:
        with (
            tc.tile_pool(name="const", bufs=1) as const,
            tc.tile_pool(name="mmps", bufs=2, space="PSUM") as mmps,
            tc.tile_pool(name="predps", bufs=1, space="PSUM") as predps,
            tc.tile_pool(name="smallps", bufs=2, space="PSUM") as smallps,
            tc.tile_pool(name="epool", bufs=4) as epool,
            tc.tile_pool(name="opool", bufs=2) as opool,
        ):
            # PE warm-up source: first DVE op so matmuls can start early
            wu_sb = const.tile([128, 512], BF16, tag="wu")
            nc.vector.memset(wu_sb, 0.0)

            # ---- input loads over 4 DGE rings ----
            wt_sb0 = const.tile([128, KPIX], BF16, tag="wt0")
            wt_sb1 = const.tile([128, KPIX], BF16, tag="wt1")
            nc.scalar.dma_start(out=wt_sb0, in_=wt_d.ap()[0:128, :])
            nc.scalar.dma_start(out=wt_sb1, in_=wt_d.ap()[128:FEAT, :])

            pt0_sb = const.tile([128, NPAT], BF16, tag="pt0")
            pt1_sb = const.tile([128, NPAT], BF16, tag="pt1")
            # sync ring: tgt block first (gt feeds every S chunk), then b0, b1
            nc.sync.dma_start(out=pt0_sb[:, 3072:4096], in_=pt0_d.ap()[:, 3072:4096])
            nc.sync.dma_start(out=pt0_sb[:, 0:1024], in_=pt0_d.ap()[:, 0:1024])
            nc.sync.dma_start(out=pt0_sb[:, 1024:2048], in_=pt0_d.ap()[:, 1024:2048])
            # scalar ring (after wT): ref b2, needed last
            nc.scalar.dma_start(out=pt0_sb[:, 2048:3072], in_=pt0_d.ap()[:, 2048:3072])
            # gpsimd ring: pt1 (64-row half) + labels
            nc.gpsimd.dma_start(out=pt1_sb[0:64, 3072:4096], in_=pt1_d.ap()[:, 3072:4096])
            nc.gpsimd.dma_start(out=pt1_sb[0:64, 0:3072], in_=pt1_d.ap()[:, 0:3072])
            lab_sb = const.tile([128, RC, LABC], BF16, tag="lab")
            nc.gpsimd.dma_start(
                out=lab_sb[:, :, 32:48],
                in_=lab_d.ap().rearrange("p (rc k) -> p rc k", k=K_LAB),
            )
            nc.gpsimd.memset(lab_sb[:, :, 0:16].bitcast(BF16), 1.0)
            nc.gpsimd.memset(lab_sb[:, :, 16:32].bitcast(BF16), 0.0)

            # on-chip zero pads + constants (DVE, after its DMA issues)
            nc.vector.memset(pt1_sb[64:128, :].bitcast(BF16), 0.0)
            g_sb0 = const.tile([128, KPIX], BF16, tag="g0")
            g_sb1 = const.tile([128, KPIX], BF16, tag="g1")
            nc.vector.memset(g_sb1[64:128, :].bitcast(BF16), 0.0)
            gt_sb = [
                const.tile([128, T], BF16, tag="gt0", name="gt_sb0"),
                const.tile([128, T], BF16, tag="gt1", name="gt_sb1"),
            ]
            nc.vector.memset(gt_sb[1][64:128, :].bitcast(BF16), 0.0)
            shift_sb = const.tile([128, 1], F32, tag="shift")
            nc.vector.memset(shift_sb, EXP_SHIFT)

            # ---- PE clock warm-up during the DMA prologue (HAM) ----
            for _ in range(N_WARMUP):
                wps = smallps.tile([128, 512], F32, tag="wp", name="wps")
                nc.tensor.matmul(wps, wu_sb[:, 0:128], wu_sb, start=True, stop=True)

            # ---- 1. G = W W^T  (contract over FEAT=256, two 128-chunks) ----
            g0_ps = smallps.tile([128, KPIX], F32, tag="wp", name="g0_ps")
            nc.tensor.matmul(g0_ps, wt_sb0[:, 0:128], wt_sb0, start=True, stop=False)
            nc.tensor.matmul(g0_ps, wt_sb1[:, 0:128], wt_sb1, start=False, stop=True)
            g1_ps = smallps.tile([128, KPIX], F32, tag="wp", name="g1_ps")
            nc.tensor.matmul(
                g1_ps[0:64, :], wt_sb0[:, 128:KPIX], wt_sb0, start=True, stop=False
            )
            nc.tensor.matmul(
                g1_ps[0:64, :], wt_sb1[:, 128:KPIX], wt_sb1, start=False, stop=True
            )
            nc.scalar.copy(g_sb0, g0_ps)
            nc.scalar.copy(g_sb1[0:64, :], g1_ps[0:64, :])

            # ---- 2. gt = G @ p_tgt  [192, 1024] in two k1-chunks ----
            for k1c in range(2):
                sz = 128 if k1c == 0 else 64
                k1sl = slice(128 * k1c, 128 * k1c + sz)
                for ph in range(2):
                    psl = slice(ph * 512, (ph + 1) * 512)
                    tsl = slice(R + ph * 512, R + (ph + 1) * 512)
                    gt_ps = smallps.tile([128, 512], F32, tag="wp", name="gt_ps")
                    nc.tensor.matmul(
                        gt_ps[0:sz, :], g_sb0[:, k1sl], pt0_sb[:, tsl],
                        start=True, stop=False,
                    )
                    nc.tensor.matmul(
                        gt_ps[0:sz, :], g_sb1[:, k1sl], pt1_sb[:, tsl],
                        start=False, stop=True,
                    )
                    dst = gt_sb[k1c][0:sz, psl]
                    if (k1c + ph) % 2 == 0:
                        nc.vector.tensor_copy(dst, gt_ps[0:sz, :])
                    else:
                        nc.scalar.copy(dst, gt_ps[0:sz, :])

            # ---- 3/4/5. S chunks -> exp -> pred accumulation ----
            pred_ps = predps.tile([LABC, T], F32, tag="pred")
            e_tiles = {}

            def s_part(rc):
                rsl = slice(rc * 128, (rc + 1) * 128)
                s_ps = mmps.tile([128, T], F32, tag="mm", name="s_ps")
                # weight reuse: pt0 chunk streams both halves, then pt1 chunk
                nc.tensor.matmul(
                    s_ps[:, 0:512], pt0_sb[:, rsl], gt_sb[0][:, 0:512],
                    start=True, stop=False,
                )
                nc.tensor.matmul(
                    s_ps[:, 512:1024], pt0_sb[:, rsl], gt_sb[0][:, 512:1024],
                    start=True, stop=False,
                )
                nc.tensor.matmul(
                    s_ps[:, 0:512], pt1_sb[:, rsl], gt_sb[1][:, 0:512],
                    start=False, stop=True,
                )
                nc.tensor.matmul(
                    s_ps[:, 512:1024], pt1_sb[:, rsl], gt_sb[1][:, 512:1024],
                    start=False, stop=True,
                )
                e_sb = epool.tile([128, T], BF16, tag="e", name="e_sb")
                nc.scalar.activation(
                    e_sb, s_ps, mybir.ActivationFunctionType.Exp,
                    bias=shift_sb, scale=1.0,
                )
                e_tiles[rc] = e_sb

            def pred_part(rc):
                e_sb = e_tiles.pop(rc)
                for th in range(2):
                    psl = slice(th * 512, (th + 1) * 512)
                    nc.tensor.matmul(
                        pred_ps[:, psl],
                        lab_sb[:, rc, :],
                        e_sb[:, psl],
                        start=(rc == 0), stop=(rc == RC - 1),
                    )

            # pred lags two chunks behind its exp so PE never waits on ACT
            PRED_LAG = 2
            for rc in range(RC):
                s_part(rc)
                if rc >= PRED_LAG:
                    pred_part(rc - PRED_LAG)
            for rc in range(RC - PRED_LAG, RC):
                pred_part(rc)

            # ---- normalize label rows by replicated denom rows ----
            rec = opool.tile([K_LAB, T], F32, tag="rec")
            nc.vector.reciprocal_approx_fast(rec, pred_ps[0:K_LAB, :])
            o_sb = opool.tile([K_LAB, T], F32, tag="o")
            nc.vector.tensor_mul(o_sb, pred_ps[32:32 + K_LAB, :], rec)
            nc.sync.dma_start(out=out_d.ap(), in_=o_sb)

    nc.compile()
    return nc


_NC_CACHE = None


def _get_nc():
    global _NC_CACHE
    if _NC_CACHE is None:
        _NC_CACHE = _build_nc()
    return _NC_CACHE


def prep_in_maps(reference_images, target_images, reference_labels, w_feat):
    """Host-side sharding + layout prep (reshape/transpose/cast only)."""
    ri = np.ascontiguousarray(reference_images, dtype=np.float32)
    ti = np.ascontiguousarray(target_images, dtype=np.float32)
    lab = np.ascontiguousarray(reference_labels, dtype=np.float32)
    wf = np.ascontiguousarray(w_feat, dtype=np.float32)

    wt = np.ascontiguousarray(wf.reshape(KPIX, FEAT).T).astype(bfloat16)
    imgs = np.concatenate([ri, ti], axis=1)  # [N, 4, H, W, C]
    # patchesT[n] : [(dy dx ch), (img py px)]
    ptT = np.ascontiguousarray(
        imgs.reshape(N, NIMG, HP, PATCH, HP, PATCH, C)
        .transpose(0, 3, 5, 6, 1, 2, 4)
        .reshape(N, KPIX, NPAT)
    ).astype(bfloat16)
    lab_sw = np.ascontiguousarray(
        lab.reshape(N, RC, 128, K_LAB).transpose(0, 2, 1, 3).reshape(N, 128, RC * K_LAB)
    ).astype(bfloat16)
    return [
        {
            "pt0": np.ascontiguousarray(ptT[n][0:128]),
            "pt1": np.ascontiguousarray(ptT[n][128:KPIX]),
            "wt": wt,
            "lab": lab_sw[n],
        }
        for n in range(N)
    ]


def run(in_maps, **kwargs):
    nc = _get_nc()
    return run_bass_kernel_spmd(nc, in_maps, list(range(N)), **kwargs)


def kernel(reference_images, target_images, reference_labels, w_feat):
    in_maps = prep_in_maps(
        reference_images, target_images, reference_labels, w_feat
    )
    res = run(in_maps)
    # device emits [16, T]; transpose to [T, 16] here (pure layout)
    out = np.stack(
        [np.ascontiguousarray(res.results[n]["out"].T) for n in range(N)]
    )
    return out.reshape(N, T_T, HP, HP, K_LAB)


# revision 5
# speedup vs baseline: 1.0907x; 1.0015x over previous
"""Trainium2 Bass kernel for nn_Colorizer (retrieval_knn).

v3 — Gram-matrix reformulation + all-bf16 (validated rel-err ~6e-3 vs
the 2e-2 gate on the fixed setup_inputs seed):

  S[r,t] = (W^T p_r)·(W^T p_t) = p_r^T (W W^T) p_t
so the per-image conv collapses to:
  1. G = W W^T            [192,192]   (768 PE cycles; W^T arrives as a
     [128, 2*192] two-chunk pack so each DMA row carries 768B — the
     naive [c,k] layout is 384B/row and descriptor-bound)
  2. gt = G @ p_tgt       [192,1024]  (4096 cycles; replaces the
     16384-cycle 4-image conv)
  3. S chunks: S[rc] = p_ref[:,rc]^T @ gt   (24 x 2048 cycles)
  4. E = exp(S - 50) on ACT in [128,512] halves (pipelines against PE;
     the -50 shift rides contract row 64 of the zero-pad region:
     pt1 row64 = 1, gt1 row64 = -50, so no bias operand; softmax is
     shift-invariant, fp32 exp can't overflow, underflow is harmless)
  5. pred rows via lab_aug = [ones;zeros;labels] matmul (denominator
     rides along as rows 0..15), normalize in T-halves overlapped with
     the last pred matmuls, DMA out [16,1024] fp32.

All matmuls bf16 (1 col/cycle at full clock). Inputs land bf16 (1.8
MB/core) over three DGE rings, ordered so every tensor arrives just
before its first use. PE work: 768 + 4096 + 24*(2048+1024) ~= 78.6k
cycles. Warm-up matmuls during the DMA prologue start the HAM clock
ramp (1.2 -> 2.4 GHz needs ~3 us of continuous PE work).

Zero-padding: contract rows 65:128 of the k=128..191 chunk are zero on
BOTH operands (0 x garbage could be NaN); row 64 carries the shift.

Host side only reshapes/transposes/casts data; all FLOPs run on device.
Built on bacc.Bacc so compile() legalizes multi-semaphore waits.
"""

import numpy as np
from ml_dtypes import bfloat16

import concourse.mybir as mybir
from concourse import bacc
from concourse.bass_utils import run_bass_kernel_spmd
from concourse.tile import TileContext

F32 = mybir.dt.float32
BF16 = mybir.dt.bfloat16

N = 8            # samples == cores
R_T, T_T = 3, 1  # ref / target frames
H = W_IMG = 256
C = 3
PATCH = 8
FEAT = 256
K_LAB = 16
HP = H // PATCH          # 32
PPI = HP * HP            # 1024 patches per image
NIMG = R_T + T_T         # 4
NPAT = NIMG * PPI        # 4096
KPIX = PATCH * PATCH * C  # 192
R = R_T * PPI            # 3072
T = T_T * PPI            # 1024
RC = R // 128            # 24 r-chunks
LABC = 48                # 16 ones cols, 16 zero cols, 16 label cols
EXP_SHIFT = -50.0
N_WARMUP = 6
HT = T // 2              # 512-col halves


def _build_nc():
    nc = bacc.Bacc(trn_type="TRN2", target_bir_lowering=False)

    pt0_d = nc.declare_dram_parameter("pt0", [128, NPAT], BF16, isOutput=False)
    pt1_d = nc.declare_dram_parameter("pt1", [64, NPAT], BF16, isOutput=False)
    # wt_pack[p, j*KPIX+k] = W^T[j*128+p, k]  (c-chunk j in {0,1})
    wt_d = nc.declare_dram_parameter("wt", [128, 2 * KPIX], BF16, isOutput=False)
    lab_d = nc.declare_dram_parameter("lab", [128, RC * K_LAB], BF16, isOutput=False)
    out_d = nc.declare_dram_parameter("out", [K_LAB, T], F32, isOutput=True)

    with TileContext(nc) as tc:
        with (
            tc.tile_pool(name="const", bufs=1) as const,
            tc.tile_pool(name="mmps", bufs=4, space="PSUM") as mmps,
            tc.tile_pool(name="predps", bufs=1, space="PSUM") as predps,
            tc.tile_pool(name="smallps", bufs=2, space="PSUM") as smallps,
            tc.tile_pool(name="epool", bufs=8) as epool,
            tc.tile_pool(name="opool", bufs=2) as opool,
        ):
            # PE warm-up source: first DVE op so matmuls can start early
            wu_sb = const.tile([128, 512], BF16, tag="wu")
            nc.vector.memset(wu_sb, 0.0)

            # ---- input loads over 3 DGE rings, earliest-need first ----
            wt_sb = const.tile([128, 2 * KPIX], BF16, tag="wt")
            pt0_sb = const.tile([128, NPAT], BF16, tag="pt0")
            pt1_sb = const.tile([128, NPAT], BF16, tag="pt1")
            lab_sb = const.tile([128, RC, LABC], BF16, tag="lab")

            # sync ring: wt lower half, then tgt halves, then ref b1, b2
            nc.sync.dma_start(out=wt_sb[64:128, :], in_=wt_d.ap()[64:128, :])
            nc.sync.dma_start(out=pt0_sb[:, 3072:3584], in_=pt0_d.ap()[:, 3072:3584])
            nc.sync.dma_start(out=pt0_sb[:, 3584:4096], in_=pt0_d.ap()[:, 3584:4096])
            nc.sync.dma_start(out=pt0_sb[:, 1024:2048], in_=pt0_d.ap()[:, 1024:2048])
            nc.sync.dma_start(out=pt0_sb[:, 2048:3072], in_=pt0_d.ap()[:, 2048:3072])
            # scalar ring: wt upper half, then ref b0 halves
            nc.scalar.dma_start(out=wt_sb[0:64, :], in_=wt_d.ap()[0:64, :])
            nc.scalar.dma_start(out=pt0_sb[:, 0:512], in_=pt0_d.ap()[:, 0:512])
            nc.scalar.dma_start(out=pt0_sb[:, 512:1024], in_=pt0_d.ap()[:, 512:1024])
            # gpsimd ring: pt1 tgt, pt1 ref b0, lab, pt1 ref b1, b2
            nc.gpsimd.dma_start(out=pt1_sb[0:64, 3072:4096], in_=pt1_d.ap()[:, 3072:4096])
            nc.gpsimd.dma_start(out=pt1_sb[0:64, 0:1024], in_=pt1_d.ap()[:, 0:1024])
            nc.gpsimd.dma_start(
                out=lab_sb[:, :, 32:48],
                in_=lab_d.ap().rearrange("p (rc k) -> p rc k", k=K_LAB),
            )
            nc.gpsimd.dma_start(out=pt1_sb[0:64, 1024:2048], in_=pt1_d.ap()[:, 1024:2048])
            nc.gpsimd.dma_start(out=pt1_sb[0:64, 2048:3072], in_=pt1_d.ap()[:, 2048:3072])
            nc.gpsimd.memset(lab_sb[:, :, 0:16].bitcast(BF16), 1.0)
            nc.gpsimd.memset(lab_sb[:, :, 16:32].bitcast(BF16), 0.0)

            # on-chip pads (DVE): rows 64:128 of the second contract chunk.
            # Row 64 carries the exp shift: pt1 row64 = 1 (ref cols only),
            # gt1 row64 = -50, so S accumulates -50 with no bias operand.
            g_sb0 = const.tile([128, KPIX], BF16, tag="g0")
            g_sb1 = const.tile([128, KPIX], BF16, tag="g1")
            gt_sb = [
                const.tile([128, T], BF16, tag="gt0", name="gt_sb0"),
                const.tile([128, T], BF16, tag="gt1", name="gt_sb1"),
            ]
            # (partition base must be 0/32/64/96: zero [64:128], then row 64)
            nc.vector.memset(pt1_sb[64:128, 0:3072].bitcast(BF16), 0.0)
            nc.vector.memset(pt1_sb[64:65, 0:3072].bitcast(BF16), 1.0)
            # tgt cols of row 64 must stay 0 (gt rhs side)
            nc.vector.memset(pt1_sb[64:128, 3072:4096].bitcast(BF16), 0.0)
            nc.vector.memset(g_sb1[64:128, :].bitcast(BF16), 0.0)
            nc.vector.memset(gt_sb[1][64:128, :].bitcast(BF16), 0.0)
            nc.vector.memset(gt_sb[1][64:65, :].bitcast(BF16), EXP_SHIFT)

            # ---- PE clock warm-up during the DMA prologue (HAM) ----
            for _ in range(N_WARMUP):
                wps = smallps.tile([128, 512], F32, tag="wp", name="wps")
                nc.tensor.matmul(wps, wu_sb[:, 0:128], wu_sb, start=True, stop=True)

            # ---- 1. G = W W^T (contract over FEAT as 2 packed 128-chunks) --
            g0_ps = smallps.tile([128, KPIX], F32, tag="wp", name="g0_ps")
            nc.tensor.matmul(
                g0_ps, wt_sb[:, 0:128], wt_sb[:, 0:KPIX], start=True, stop=False
            )
            nc.tensor.matmul(
                g0_ps, wt_sb[:, KPIX:KPIX + 128], wt_sb[:, KPIX:2 * KPIX],
                start=False, stop=True,
            )
            g1_ps = smallps.tile([128, KPIX], F32, tag="wp", name="g1_ps")
            nc.tensor.matmul(
                g1_ps[0:64, :], wt_sb[:, 128:KPIX], wt_sb[:, 0:KPIX],
                start=True, stop=False,
            )
            nc.tensor.matmul(
                g1_ps[0:64, :], wt_sb[:, KPIX + 128:2 * KPIX], wt_sb[:, KPIX:2 * KPIX],
                start=False, stop=True,
            )
            nc.scalar.copy(g_sb0, g0_ps)
            nc.scalar.copy(g_sb1[0:64, :], g1_ps[0:64, :])

            # ---- 2. gt = G @ p_tgt  [192, 1024] in two k1-chunks ----
            for ph in range(2):
                psl = slice(ph * HT, (ph + 1) * HT)
                tsl = slice(R + ph * HT, R + (ph + 1) * HT)
                for k1c in range(2):
                    sz = 128 if k1c == 0 else 64
                    k1sl = slice(128 * k1c, 128 * k1c + sz)
                    gt_ps = smallps.tile([128, HT], F32, tag="wp", name="gt_ps")
                    nc.tensor.matmul(
                        gt_ps[0:sz, :], g_sb0[:, k1sl], pt0_sb[:, tsl],
                        start=True, stop=False,
                    )
                    nc.tensor.matmul(
                        gt_ps[0:sz, :], g_sb1[:, k1sl], pt1_sb[:, tsl],
                        start=False, stop=True,
                    )
                    dst = gt_sb[k1c][0:sz, psl]
                    if k1c == 0:
                        nc.vector.tensor_copy(dst, gt_ps[0:sz, :])
                    else:
                        nc.scalar.copy(dst, gt_ps[0:sz, :])

            # ---- 3/4/5. S halves -> exp halves -> pred accumulation ----
            pred_ps = predps.tile([LABC, T], F32, tag="pred")
            e_tiles = {}

            def s_part(rc):
                rsl = slice(rc * 128, (rc + 1) * 128)
                for ph in range(2):
                    psl = slice(ph * HT, (ph + 1) * HT)
                    s_ps = mmps.tile([128, HT], F32, tag="mm", name="s_ps")
                    nc.tensor.matmul(
                        s_ps, pt0_sb[:, rsl], gt_sb[0][:, psl],
                        start=True, stop=False,
                    )
                    nc.tensor.matmul(
                        s_ps, pt1_sb[:, rsl], gt_sb[1][:, psl],
                        start=False, stop=True,
                    )
                    e_sb = epool.tile([128, HT], BF16, tag="e", name="e_sb")
                    nc.scalar.activation(
                        e_sb, s_ps, mybir.ActivationFunctionType.Exp, scale=1.0
                    )
                    e_tiles[(rc, ph)] = e_sb

            def pred_part(rc):
                for ph in range(2):
                    psl = slice(ph * HT, (ph + 1) * HT)
                    e_sb = e_tiles.pop((rc, ph))
                    nc.tensor.matmul(
                        pred_ps[:, psl],
                        lab_sb[:, rc, :],
                        e_sb,
                        start=(rc == 0), stop=(rc == RC - 1),
                    )

            # pred lags two chunks behind its exp so PE never waits on ACT
            PRED_LAG = 2
            for rc in range(RC):
                s_part(rc)
                if rc >= PRED_LAG:
                    pred_part(rc - PRED_LAG)
            for rc in range(RC - PRED_LAG, RC):
                pred_part(rc)

            # ---- normalize label rows in T-halves (overlaps pred drain) ----
            for ph in range(2):
                psl = slice(ph * HT, (ph + 1) * HT)
                rec = opool.tile([K_LAB, HT], F32, tag="rec", name="rec")
                nc.vector.reciprocal_approx_fast(rec, pred_ps[0:K_LAB, psl])
                o_sb = opool.tile([K_LAB, HT], F32, tag="o", name="o_sb")
                nc.vector.tensor_mul(o_sb, pred_ps[32:32 + K_LAB, psl], rec)
                nc.sync.dma_start(out=out_d.ap()[:, psl], in_=o_sb)

    nc.compile()
    return nc


_NC_CACHE = None


def _get_nc():
    global _NC_CACHE
    if _NC_CACHE is None:
        _NC_CACHE = _build_nc()
    return _NC_CACHE


def prep_in_maps(reference_images, target_images, reference_labels, w_feat):
    """Host-side sharding + layout prep (reshape/transpose/cast only)."""
    ri = np.ascontiguousarray(reference_images, dtype=np.float32)
    ti = np.ascontiguousarray(target_images, dtype=np.float32)
    lab = np.ascontiguousarray(reference_labels, dtype=np.float32)
    wf = np.ascontiguousarray(w_feat, dtype=np.float32)

    wT = np.ascontiguousarray(wf.reshape(KPIX, FEAT).T)       # [256, 192]
    wt_pack = np.concatenate([wT[0:128], wT[128:256]], axis=1)  # [128, 384]
    wt_pack = np.ascontiguousarray(wt_pack).astype(bfloat16)
    imgs = np.concatenate([ri, ti], axis=1)  # [N, 4, H, W, C]
    # patchesT[n] : [(dy dx ch), (img py px)]
    ptT = np.ascontiguousarray(
        imgs.reshape(N, NIMG, HP, PATCH, HP, PATCH, C)
        .transpose(0, 3, 5, 6, 1, 2, 4)
        .reshape(N, KPIX, NPAT)
    ).astype(bfloat16)
    lab_sw = np.ascontiguousarray(
        lab.reshape(N, RC, 128, K_LAB).transpose(0, 2, 1, 3).reshape(N, 128, RC * K_LAB)
    ).astype(bfloat16)
    return [
        {
            "pt0": np.ascontiguousarray(ptT[n][0:128]),
            "pt1": np.ascontiguousarray(ptT[n][128:KPIX]),
            "wt": wt_pack,
            "lab": lab_sw[n],
        }
        for n in range(N)
    ]


def run(in_maps, **kwargs):
    nc = _get_nc()
    return run_bass_kernel_spmd(nc, in_maps, list(range(N)), **kwargs)


def kernel(reference_images, target_images, reference_labels, w_feat):
    in_maps = prep_in_maps(
        reference_images, target_images, reference_labels, w_feat
    )
    res = run(in_maps)
    # device emits [16, T]; transpose to [T, 16] here (pure layout)
    out = np.stack(
        [np.ascontiguousarray(res.results[n]["out"].T) for n in range(N)]
    )
    return out.reshape(N, T_T, HP, HP, K_LAB)


# revision 6
# speedup vs baseline: 1.1663x; 1.0692x over previous
"""Trainium2 Bass kernel for nn_Colorizer (retrieval_knn).

v4 — Gram-matrix reformulation + all-bf16 (validated rel-err ~6e-3 vs
the 2e-2 gate on the fixed setup_inputs seed):

  S[r,t] = (W^T p_r)·(W^T p_t) = p_r^T (W W^T) p_t
so the per-image conv collapses to:
  1. G = W W^T            [192,192]   (768 PE cycles; W^T arrives as a
     [128, 2*192] two-chunk pack so each DMA row carries 768B — the
     naive [c,k] layout is 384B/row and descriptor-bound, ~3.2us/tile)
  2. gt = G @ p_tgt       [192,1024]  (4096 cycles; replaces the
     16384-cycle 4-image conv of the v1 kernel)
  3. S chunks: S[rc] = p_ref[:,rc]^T @ gt   (24 x 2048 cycles)
  4. E = exp(S - 50) on ACT, full [128,1024] tiles (one ACTIVATE costs
     ~260ns fixed + ~0.85ns/elem, so halves are net slower); -50 bias
     keeps fp32 exp in range, softmax is shift-invariant
  5. pred rows via lab_aug = [ones;zeros;labels] matmul (denominator
     rides along as rows 0..15), normalize in T-halves, DMA out fp32.

All matmuls bf16 (1 col/cycle at full clock). Inputs land bf16 (1.8
MB/core) over three DGE rings, ordered so every tensor arrives just
before its first use. PE work: 768 + 4096 + 24*(2048+1024) ~= 78.6k
cycles ~= 33 us at 2.4 GHz. Warm-up matmuls during the DMA prologue
start the HAM clock ramp (1.2 -> 2.4 GHz needs ~3us continuous work).

PSUM: one 3-buf pool of [128,1024] fp32 slots (6 banks) rotates
through warm-up/G/gt and the 24 S tiles — 3 chunks of lookahead so the
PE never stalls on ACT freeing a slot — plus 2 banks for pred.

Zero-padding: contract rows 64:128 of the k=128..191 chunk are zeroed
on BOTH operands (0 x garbage could be NaN). DVE ops are few and fat:
per-op fixed cost is ~0.5-0.7us, so pads are merged, not split.

Host side only reshapes/transposes/casts data; all FLOPs run on device.
Built on bacc.Bacc so compile() legalizes multi-semaphore waits.
"""

import numpy as np
from ml_dtypes import bfloat16

import concourse.mybir as mybir
from concourse import bacc
from concourse.bass_utils import run_bass_kernel_spmd
from concourse.tile import TileContext

F32 = mybir.dt.float32
BF16 = mybir.dt.bfloat16

N = 8            # samples == cores
R_T, T_T = 3, 1  # ref / target frames
H = W_IMG = 256
C = 3
PATCH = 8
FEAT = 256
K_LAB = 16
HP = H // PATCH          # 32
PPI = HP * HP            # 1024 patches per image
NIMG = R_T + T_T         # 4
NPAT = NIMG * PPI        # 4096
KPIX = PATCH * PATCH * C  # 192
R = R_T * PPI            # 3072
T = T_T * PPI            # 1024
RC = R // 128            # 24 r-chunks
LABC = 48                # 16 ones cols, 16 zero cols, 16 label cols
EXP_SHIFT = -50.0
N_WARMUP = 6
HT = T // 2              # 512-col halves


def _build_nc():
    nc = bacc.Bacc(trn_type="TRN2", target_bir_lowering=False)

    pt0_d = nc.declare_dram_parameter("pt0", [128, NPAT], BF16, isOutput=False)
    pt1_d = nc.declare_dram_parameter("pt1", [64, NPAT], BF16, isOutput=False)
    # wt_pack[p, j*KPIX+k] = W^T[j*128+p, k]  (c-chunk j in {0,1})
    wt_d = nc.declare_dram_parameter("wt", [128, 2 * KPIX], BF16, isOutput=False)
    lab_d = nc.declare_dram_parameter("lab", [128, RC * K_LAB], BF16, isOutput=False)
    out_d = nc.declare_dram_parameter("out", [K_LAB, T], F32, isOutput=True)

    with TileContext(nc) as tc:
        with (
            tc.tile_pool(name="const", bufs=1) as const,
            tc.tile_pool(name="mmps", bufs=3, space="PSUM") as mmps,
            tc.tile_pool(name="predps", bufs=1, space="PSUM") as predps,
            tc.tile_pool(name="epool", bufs=4) as epool,
            tc.tile_pool(name="opool", bufs=2) as opool,
        ):
            # PE warm-up source: first DVE op so matmuls can start early
            wu_sb = const.tile([128, 512], BF16, tag="wu")
            nc.vector.memset(wu_sb, 0.0)

            # ---- input loads over 3 DGE rings, earliest-need first ----
            wt_sb = const.tile([128, 2 * KPIX], BF16, tag="wt")
            pt0_sb = const.tile([128, NPAT], BF16, tag="pt0")
            pt1_sb = const.tile([128, NPAT], BF16, tag="pt1")
            lab_sb = const.tile([128, RC, LABC], BF16, tag="lab")

            # sync ring: wt lower half, then tgt halves, then ref b1, b2
            nc.sync.dma_start(out=wt_sb[64:128, :], in_=wt_d.ap()[64:128, :])
            nc.sync.dma_start(out=pt0_sb[:, 3072:3584], in_=pt0_d.ap()[:, 3072:3584])
            nc.sync.dma_start(out=pt0_sb[:, 3584:4096], in_=pt0_d.ap()[:, 3584:4096])
            nc.sync.dma_start(out=pt0_sb[:, 1024:2048], in_=pt0_d.ap()[:, 1024:2048])
            nc.sync.dma_start(out=pt0_sb[:, 2048:3072], in_=pt0_d.ap()[:, 2048:3072])
            # scalar ring: wt upper half, then ref b0 halves
            nc.scalar.dma_start(out=wt_sb[0:64, :], in_=wt_d.ap()[0:64, :])
            nc.scalar.dma_start(out=pt0_sb[:, 0:512], in_=pt0_d.ap()[:, 0:512])
            nc.scalar.dma_start(out=pt0_sb[:, 512:1024], in_=pt0_d.ap()[:, 512:1024])
            # gpsimd ring: pt1 tgt, pt1 ref b0, lab, pt1 ref b1, b2
            nc.gpsimd.dma_start(out=pt1_sb[0:64, 3072:4096], in_=pt1_d.ap()[:, 3072:4096])
            nc.gpsimd.dma_start(out=pt1_sb[0:64, 0:1024], in_=pt1_d.ap()[:, 0:1024])
            nc.gpsimd.dma_start(
                out=lab_sb[:, :, 32:48],
                in_=lab_d.ap().rearrange("p (rc k) -> p rc k", k=K_LAB),
            )
            nc.gpsimd.dma_start(out=pt1_sb[0:64, 1024:2048], in_=pt1_d.ap()[:, 1024:2048])
            nc.gpsimd.dma_start(out=pt1_sb[0:64, 2048:3072], in_=pt1_d.ap()[:, 2048:3072])
            nc.gpsimd.memset(lab_sb[:, :, 0:16].bitcast(BF16), 1.0)
            nc.gpsimd.memset(lab_sb[:, :, 16:32].bitcast(BF16), 0.0)

            # G rows 128:192 and gt rows 128:192 live in one tile so the
            # shared 64:128 zero-pad is a single DVE memset
            gg_sb = const.tile([128, KPIX + T], BF16, tag="gg")
            g_sb1 = gg_sb[:, 0:KPIX]
            gt_sb1 = gg_sb[:, KPIX:KPIX + T]
            g_sb0 = const.tile([128, KPIX], BF16, tag="g0")
            gt_sb0 = const.tile([128, T], BF16, tag="gt0")

            # DVE pad chain, need-ordered: gg (gt kc1 ~11.5), pt1 tgt pad
            # (gt kc1 rhs), pt1 b0 pad (S kc1 rc0 ~13), rest on gpsimd
            nc.vector.memset(gg_sb[64:128, :].bitcast(BF16), 0.0)
            nc.vector.memset(pt1_sb[64:128, 3072:4096].bitcast(BF16), 0.0)
            nc.vector.memset(pt1_sb[64:128, 0:1024].bitcast(BF16), 0.0)
            shift_sb = const.tile([128, 1], F32, tag="shift")
            nc.vector.memset(shift_sb, EXP_SHIFT)
            nc.gpsimd.memset(pt1_sb[64:128, 1024:3072].bitcast(BF16), 0.0)

            # ---- PE clock warm-up during the DMA prologue (HAM) ----
            for _ in range(N_WARMUP):
                wps = mmps.tile([128, 512], F32, tag="mm", name="wps")
                nc.tensor.matmul(wps, wu_sb[:, 0:128], wu_sb, start=True, stop=True)

            # ---- 1. G = W W^T (contract over FEAT as 2 packed 128-chunks) --
            g0_ps = mmps.tile([128, KPIX], F32, tag="mm", name="g0_ps")
            nc.tensor.matmul(
                g0_ps, wt_sb[:, 0:128], wt_sb[:, 0:KPIX], start=True, stop=False
            )
            nc.tensor.matmul(
                g0_ps, wt_sb[:, KPIX:KPIX + 128], wt_sb[:, KPIX:2 * KPIX],
                start=False, stop=True,
            )
            g1_ps = mmps.tile([128, KPIX], F32, tag="mm", name="g1_ps")
            nc.tensor.matmul(
                g1_ps[0:64, :], wt_sb[:, 128:KPIX], wt_sb[:, 0:KPIX],
                start=True, stop=False,
            )
            nc.tensor.matmul(
                g1_ps[0:64, :], wt_sb[:, KPIX + 128:2 * KPIX], wt_sb[:, KPIX:2 * KPIX],
                start=False, stop=True,
            )
            nc.scalar.copy(g_sb0, g0_ps)
            nc.scalar.copy(g_sb1[0:64, :], g1_ps[0:64, :])

            # ---- 2. gt = G @ p_tgt  [192, 1024] in two k1-chunks ----
            for ph in range(2):
                psl = slice(ph * HT, (ph + 1) * HT)
                tsl = slice(R + ph * HT, R + (ph + 1) * HT)
                for k1c in range(2):
                    sz = 128 if k1c == 0 else 64
                    k1sl = slice(128 * k1c, 128 * k1c + sz)
                    gt_ps = mmps.tile([128, HT], F32, tag="mm", name="gt_ps")
                    nc.tensor.matmul(
                        gt_ps[0:sz, :], g_sb0[:, k1sl], pt0_sb[:, tsl],
                        start=True, stop=False,
                    )
                    nc.tensor.matmul(
                        gt_ps[0:sz, :], g_sb1[:, k1sl], pt1_sb[:, tsl],
                        start=False, stop=True,
                    )
                    dst = (gt_sb0 if k1c == 0 else gt_sb1)[0:sz, psl]
                    if k1c == 0:
                        nc.vector.tensor_copy(dst, gt_ps[0:sz, :])
                    else:
                        nc.scalar.copy(dst, gt_ps[0:sz, :])

            # ---- 3/4/5. S chunks -> exp -> pred accumulation ----
            pred_ps = predps.tile([LABC, T], F32, tag="pred")
            e_tiles = {}

            def s_part(rc):
                rsl = slice(rc * 128, (rc + 1) * 128)
                s_ps = mmps.tile([128, T], F32, tag="mm", name="s_ps")
                # weight reuse: pt0 chunk streams both halves, then pt1 chunk
                nc.tensor.matmul(
                    s_ps[:, 0:HT], pt0_sb[:, rsl], gt_sb0[:, 0:HT],
                    start=True, stop=False,
                )
                nc.tensor.matmul(
                    s_ps[:, HT:T], pt0_sb[:, rsl], gt_sb0[:, HT:T],
                    start=True, stop=False,
                )
                nc.tensor.matmul(
                    s_ps[:, 0:HT], pt1_sb[:, rsl], gt_sb1[:, 0:HT],
                    start=False, stop=True,
                )
                nc.tensor.matmul(
                    s_ps[:, HT:T], pt1_sb[:, rsl], gt_sb1[:, HT:T],
                    start=False, stop=True,
                )
                e_sb = epool.tile([128, T], BF16, tag="e", name="e_sb")
                nc.scalar.activation(
                    e_sb, s_ps, mybir.ActivationFunctionType.Exp,
                    bias=shift_sb, scale=1.0,
                )
                e_tiles[rc] = e_sb

            def pred_part(rc):
                e_sb = e_tiles.pop(rc)
                for th in range(2):
                    psl = slice(th * HT, (th + 1) * HT)
                    nc.tensor.matmul(
                        pred_ps[:, psl],
                        lab_sb[:, rc, :],
                        e_sb[:, psl],
                        start=(rc == 0), stop=(rc == RC - 1),
                    )

            # pred lags two chunks behind its exp so PE never waits on ACT
            PRED_LAG = 2
            for rc in range(RC):
                s_part(rc)
                if rc >= PRED_LAG:
                    pred_part(rc - PRED_LAG)
            for rc in range(RC - PRED_LAG, RC):
                pred_part(rc)

            # ---- normalize label rows in T-halves (overlaps pred drain) ----
            for ph in range(2):
                psl = slice(ph * HT, (ph + 1) * HT)
                rec = opool.tile([K_LAB, HT], F32, tag="rec", name="rec")
                nc.vector.reciprocal_approx_fast(rec, pred_ps[0:K_LAB, psl])
                o_sb = opool.tile([K_LAB, HT], F32, tag="o", name="o_sb")
                nc.vector.tensor_mul(o_sb, pred_ps[32:32 + K_LAB, psl], rec)
                nc.sync.dma_start(out=out_d.ap()[:, psl], in_=o_sb)

    nc.compile()
    return nc


_NC_CACHE = None


def _get_nc():
    global _NC_CACHE
    if _NC_CACHE is None:
        _NC_CACHE = _build_nc()
    return _NC_CACHE


def prep_in_maps(reference_images, target_images, reference_labels, w_feat):
    """Host-side sharding + layout prep (reshape/transpose/cast only)."""
    ri = np.ascontiguousarray(reference_images, dtype=np.float32)
    ti = np.ascontiguousarray(target_images, dtype=np.float32)
    lab = np.ascontiguousarray(reference_labels, dtype=np.float32)
    wf = np.ascontiguousarray(w_feat, dtype=np.float32)

    wT = np.ascontiguousarray(wf.reshape(KPIX, FEAT).T)       # [256, 192]
    wt_pack = np.concatenate([wT[0:128], wT[128:256]], axis=1)  # [128, 384]
    wt_pack = np.ascontiguousarray(wt_pack).astype(bfloat16)
    imgs = np.concatenate([ri, ti], axis=1)  # [N, 4, H, W, C]
    # patchesT[n] : [(dy dx ch), (img py px)]
    ptT = np.ascontiguousarray(
        imgs.reshape(N, NIMG, HP, PATCH, HP, PATCH, C)
        .transpose(0, 3, 5, 6, 1, 2, 4)
        .reshape(N, KPIX, NPAT)
    ).astype(bfloat16)
    lab_sw = np.ascontiguousarray(
        lab.reshape(N, RC, 128, K_LAB).transpose(0, 2, 1, 3).reshape(N, 128, RC * K_LAB)
    ).astype(bfloat16)
    return [
        {
            "pt0": np.ascontiguousarray(ptT[n][0:128]),
            "pt1": np.ascontiguousarray(ptT[n][128:KPIX]),
            "wt": wt_pack,
            "lab": lab_sw[n],
        }
        for n in range(N)
    ]


def run(in_maps, **kwargs):
    nc = _get_nc()
    return run_bass_kernel_spmd(nc, in_maps, list(range(N)), **kwargs)


def kernel(reference_images, target_images, reference_labels, w_feat):
    in_maps = prep_in_maps(
        reference_images, target_images, reference_labels, w_feat
    )
    res = run(in_maps)
    # device emits [16, T]; transpose to [T, 16] here (pure layout)
    out = np.stack(
        [np.ascontiguousarray(res.results[n]["out"].T) for n in range(N)]
    )
    return out.reshape(N, T_T, HP, HP, K_LAB)
